# revision 18
# baseline (speedup 1.0000x reference)
"""DeltaNet fused kernel for 8 TRN2 NeuronCores (Bass/Tile).

Math (reference):
    s  = x @ W_slow_w.T + W_slow_b            [B, 3073]
    k  = s[:, :1024]; v = s[:, 1024:2048]; q = s[:, 2048:3072]
    lr = sigmoid(s[:, 3072])
    v_bar = softmax(k) @ W_fast_w.T + W_fast_b          (W_fast_w == 0 -> v_bar = W_fast_b)
    deltaT[h,o] = sum_b sigmoid(k)[b,h] * (lr*(v - v_bar))[b,o] / B
    out = softmax(q) @ (W_fast_w + delta).T + W_fast_b

Sharding: data-parallel over batch (2048 rows/core). deltaT partials are
AllReduced across the 8 cores (two AllReduces, one per batch half, so the
first overlaps the second half's compute).

Device layout trick: all matmuls use natural layouts (no on-chip transposes):
  - host pre-transposes x (per-shard) and W_slow_w to bf16
  - q is computed TRANSPOSED (qT[h,b]) so exp(qT) is directly the lhsT of the
    final matmul; softmax denominator comes from a ones-vector matmul column
    and is applied as a per-partition output scale.
"""

import os
import sys

for _p in ("/opt/trn_rl_repo", "/root/.axon_site/_ro/trn_rl_repo"):
    if os.path.isdir(_p) and _p not in sys.path:
        sys.path.append(_p)

import numpy as np
import ml_dtypes

BF16 = ml_dtypes.bfloat16

N_CORES = 8
B_FULL = 16384
DIM = 1024          # dim_in == dim_out == dim_hidden
SLOW_OUT = 3 * DIM + 1
P = 128
NT = DIM // P       # 8 tiles along any 1024 dim


def _build_program(b_core: int, n_cores: int = N_CORES):
    """Build the SPMD Bass program (same program on every core)."""
    import concourse.bass as bass
    import concourse.mybir as mybir
    import concourse.tile as tile
    from concourse import bacc

    f32 = mybir.dt.float32
    bf16 = mybir.dt.bfloat16
    AF = mybir.ActivationFunctionType
    ALU = mybir.AluOpType

    assert b_core % (2 * P) == 0
    nbt = b_core // P                 # b-tiles per core
    # asymmetric halves: the larger second half's q-phase hides the second
    # AllReduce's ~50us cost
    nbt1 = max(2, nbt // 4)
    nbt2 = nbt - nbt1
    bh = [nbt1 * P, nbt2 * P]
    off = [0, nbt1 * P]
    nbt_h = [nbt1, nbt2]

    def chunks(width):
        lo = 0
        while lo < width:
            hi = min(lo + 512, width)
            yield lo, hi
            lo = hi

    nc = bacc.Bacc(
        "TRN2",
        target_bir_lowering=False,
        debug=False,
        num_devices=n_cores,
    )

    # ---- kernel I/O ----
    xT_h = nc.dram_tensor("xT", [DIM, b_core], bf16, kind="ExternalInput")
    wT_h = nc.dram_tensor("wT", [DIM, SLOW_OUT], bf16, kind="ExternalInput")
    bk_h = nc.dram_tensor("bk", [DIM], f32, kind="ExternalInput")     # W_slow_b[:1024]
    bvc_h = nc.dram_tensor("bvc", [DIM], f32, kind="ExternalInput")   # W_slow_b[1024:2048] - W_fast_b
    bq_h = nc.dram_tensor("bq", [DIM], f32, kind="ExternalInput")     # W_slow_b[2048:3072]
    blr_h = nc.dram_tensor("blr", [1], f32, kind="ExternalInput")     # W_slow_b[3072]
    wfb_h = nc.dram_tensor("wfb", [DIM], f32, kind="ExternalInput")   # W_fast_b
    out_h = nc.dram_tensor("out", [b_core, DIM], f32, kind="ExternalOutput")

    inv_b = 1.0 / float(b_core * n_cores)

    with tile.TileContext(nc) as tc:
        with (
            tc.tile_pool(name="persist", bufs=1) as persist,
            tc.tile_pool(name="psum", bufs=8, space="PSUM") as psum,
            tc.tile_pool(name="tmp", bufs=4) as tmp,
            tc.tile_pool(name="small", bufs=6) as small,
            tc.tile_pool(name="ost", bufs=2) as ost,
            tc.tile_pool(name="dst", bufs=2) as dstp,
            tc.tile_pool(name="arl", bufs=2) as arl,
            tc.tile_pool(name="dram", bufs=1, space="DRAM") as dram,
        ):
            # ---- persistent SBUF tensors ----
            # wT split into separate tiles per column group so the first
            # s-matmuls only depend on their own group's loads (Tile tracks
            # dependencies per tile object)
            wk = [persist.tile([P, NT, 512], bf16, name=f"wk{c}") for c in range(2)]
            wv = [persist.tile([P, NT, 512], bf16, name=f"wv{c}") for c in range(2)]
            wlr = persist.tile([P, NT, 1], bf16, name="wlr")
            wq = persist.tile([P, NT, DIM], bf16, name="wq")
            xt = persist.tile([P, NT, max(bh)], bf16, name="xt")     # one half at a time
            et = persist.tile([P, NT, b_core], bf16, name="et")      # exp(qT), full batch
            sigk = persist.tile([P, max(nbt_h), DIM], bf16, name="sigk")
            u = persist.tile([P, max(nbt_h), DIM], bf16, name="u")
            wn = persist.tile([P, NT, DIM], bf16, name="wn")         # W_new.T
            bk_b = persist.tile([P, DIM], f32, name="bk_b")
            bvc_b = persist.tile([P, DIM], f32, name="bvc_b")
            wfb_b = persist.tile([P, DIM], f32, name="wfb_b")
            bq_c = persist.tile([P, NT], f32, name="bq_c")
            blr_c = persist.tile([P, 1], f32, name="blr_c")
            ones = persist.tile([P, 1], bf16, name="ones")

            # ---- DRAM bounce buffers for the two AllReduces ----
            ar_in = [
                dram.tile([DIM, DIM], bf16, name=f"ar_in{h}") for h in range(2)
            ]
            ar_out = [
                dram.tile([DIM, DIM], bf16, name=f"ar_out{h}", addr_space="Shared")
                for h in range(2)
            ]

            # ---- constants / weights ----
            nc.vector.memset(ones[:], 1.0)
            # broadcast biases across partitions via K=1 ones-matmuls (PE is
            # idle at startup; saves 1.5 MiB of broadcast-DMA in the critical
            # startup window)
            ones_row = persist.tile([1, P], f32, name="ones_row")
            nc.vector.memset(ones_row[:], 1.0)
            for bi, (bias_dst, bias_src) in enumerate(
                ((bk_b, bk_h), (bvc_b, bvc_h), (wfb_b, wfb_h))
            ):
                for c in range(2):
                    brow = tmp.tile([1, 512], f32, tag="kv", name=f"br{bi}_{c}")
                    nc.gpsimd.dma_start(
                        out=brow[:],
                        in_=bass.AP(tensor=bias_src, offset=c * 512,
                                    ap=[[0, 1], [1, 512]]),
                    )
                    pb = psum.tile([P, 512], f32, tag="ps", name=f"pb{bi}_{c}")
                    nc.tensor.matmul(
                        pb[:], ones_row[:, :], brow[:], start=True, stop=True
                    )
                    nc.vector.tensor_copy(bias_dst[:, c * 512:(c + 1) * 512], pb[:])
            # bq_c[p, i] = bq[i*128 + p]
            nc.gpsimd.dma_start(
                out=bq_c[:],
                in_=bass.AP(tensor=bq_h, offset=0, ap=[[1, P], [P, NT]]),
            )
            nc.gpsimd.dma_start(
                out=blr_c[:],
                in_=bass.AP(tensor=blr_h, offset=0, ap=[[0, P], [1, 1]]),
            )
            # xT (half 0) first so the s-phase can start ASAP, then wT groups
            # in consumption order (lr column first: tiny).
            for i in range(NT):
                nc.sync.dma_start(
                    out=xt[:, i, 0:bh[0]], in_=xT_h[i * P:(i + 1) * P, 0:bh[0]]
                )
            for c in range(2):
                for i in range(NT):
                    nc.sync.dma_start(
                        out=wk[c][:, i, :],
                        in_=wT_h[i * P:(i + 1) * P, c * 512:(c + 1) * 512],
                    )
            for i in range(NT):
                nc.sync.dma_start(
                    out=wlr[:, i, :], in_=wT_h[i * P:(i + 1) * P, 3 * DIM:SLOW_OUT]
                )
            for c in range(2):
                for i in range(NT):
                    nc.sync.dma_start(
                        out=wv[c][:, i, :],
                        in_=wT_h[i * P:(i + 1) * P, DIM + c * 512:DIM + (c + 1) * 512],
                    )
            for i in range(NT):
                nc.sync.dma_start(
                    out=wq[:, i, :], in_=wT_h[i * P:(i + 1) * P, 2 * DIM:3 * DIM]
                )

            def emit_s(half):
                """k / v / lr per b-tile; fills sigk and u for this half."""
                for t in range(nbt_h[half]):
                    ps = [
                        psum.tile([P, 512], f32, tag="ps", name=f"ps{half}_{t}_{c}")
                        for c in range(4)
                    ]
                    plr = psum.tile([P, 1], f32, tag="ps", name=f"plr{half}_{t}")
                    for i in range(NT):
                        lhs = xt[:, i, t * P:(t + 1) * P]
                        st = dict(start=(i == 0), stop=(i == NT - 1))
                        for c in range(2):
                            nc.tensor.matmul(ps[c][:], lhs, wk[c][:, i, :], **st)
                        for c in range(2):
                            nc.tensor.matmul(ps[2 + c][:], lhs, wv[c][:, i, :], **st)
                        nc.tensor.matmul(plr[:], lhs, wlr[:, i, :], **st)
                    # lr = sigmoid(plr + blr) / B
                    lr_s = small.tile([P, 1], f32, tag="lr", name=f"lr{half}_{t}")
                    nc.scalar.activation(lr_s[:], plr[:], AF.Sigmoid, bias=blr_c[:, 0:1])
                    nc.vector.tensor_scalar_mul(lr_s[:], lr_s[:], inv_b)
                    for c in range(2):
                        # sigk = sigmoid(k + bk)
                        ktmp = tmp.tile([P, 512], f32, tag="kv", name=f"kt{half}_{t}_{c}")
                        nc.vector.tensor_add(ktmp[:], ps[c][:], bk_b[:, c * 512:(c + 1) * 512])
                        nc.scalar.activation(
                            sigk[:, t, c * 512:(c + 1) * 512], ktmp[:], AF.Sigmoid
                        )
                        # u = lr/B * (v + (bv - wfb))
                        vtmp = tmp.tile([P, 512], f32, tag="kv", name=f"vt{half}_{t}_{c}")
                        nc.vector.tensor_add(
                            vtmp[:], ps[2 + c][:], bvc_b[:, c * 512:(c + 1) * 512]
                        )
                        nc.scalar.activation(
                            u[:, t, c * 512:(c + 1) * 512], vtmp[:], AF.Copy,
                            scale=lr_s[:],
                        )

            def emit_delta(half):
                """deltaT_half[h, o] = sum_b sigk * u, staged + AllReduced.

                For the second half the first AllReduce's output is folded
                into the input (scaled by 1/n_cores so the sum over cores
                adds it exactly once) — W_newT is then just ar_out[1]."""
                nb = nbt_h[half]
                for hh in range(NT):
                    pd = [
                        psum.tile([P, 512], f32, tag="ps", name=f"pd{half}_{hh}_{oc}")
                        for oc in range(2)
                    ]
                    for t in range(nb):
                        st = dict(start=(t == 0), stop=(t == nb - 1))
                        lhs = sigk[:, t, hh * P:(hh + 1) * P]
                        for oc in range(2):
                            nc.tensor.matmul(
                                pd[oc][:], lhs, u[:, t, oc * 512:(oc + 1) * 512], **st
                            )
                    dstage = dstp.tile([P, DIM], bf16, tag="ds", name=f"ds{half}_{hh}")
                    if half == 0:
                        for oc in range(2):
                            nc.vector.tensor_copy(
                                dstage[:, oc * 512:(oc + 1) * 512], pd[oc][:]
                            )
                    else:
                        a0 = arl.tile([P, DIM], bf16, tag="ar", name=f"a0_{hh}")
                        nc.sync.dma_start(
                            out=a0[:], in_=ar_out[0][hh * P:(hh + 1) * P, :]
                        )
                        for oc in range(2):
                            nc.vector.scalar_tensor_tensor(
                                dstage[:, oc * 512:(oc + 1) * 512],
                                a0[:, oc * 512:(oc + 1) * 512],
                                1.0 / n_cores,
                                pd[oc][:],
                                op0=ALU.mult,
                                op1=ALU.add,
                            )
                    nc.sync.dma_start(
                        out=ar_in[half][hh * P:(hh + 1) * P, :], in_=dstage[:]
                    )
                nc.gpsimd.collective_compute(
                    "AllReduce",
                    mybir.AluOpType.add,
                    replica_groups=[list(range(n_cores))],
                    ins=[ar_in[half][:, :]],
                    outs=[ar_out[half][:, :]],
                )
                if half == 1:
                    for hh in range(NT):
                        nc.sync.dma_start(
                            out=wn[:, hh, :], in_=ar_out[1][hh * P:(hh + 1) * P, :]
                        )

            def emit_q(half):
                """et = exp(qT + bq) (transposed layout)."""
                for hh in range(NT):
                    for lo, hi in chunks(bh[half]):
                        w = hi - lo
                        pq = psum.tile(
                            [P, 512], f32, tag="ps", name=f"pq{half}_{hh}_{lo}"
                        )
                        for i in range(NT):
                            nc.tensor.matmul(
                                pq[:, 0:w],
                                wq[:, i, hh * P:(hh + 1) * P],
                                xt[:, i, lo:hi],
                                start=(i == 0),
                                stop=(i == NT - 1),
                            )
                        nc.scalar.activation(
                            et[:, hh, off[half] + lo:off[half] + hi],
                            pq[:, 0:w],
                            AF.Exp,
                            bias=bq_c[:, hh:hh + 1],
                        )

            # half 0: q before delta so the delta matmuls cover the xT reload;
            # half 1: delta first so AR2 overlaps the (large) q2-phase.
            emit_s(0)
            emit_q(0)
            emit_delta(0)
            for i in range(NT):
                nc.sync.dma_start(
                    out=xt[:, i, 0:bh[1]],
                    in_=xT_h[i * P:(i + 1) * P, off[1]:off[1] + bh[1]],
                )
            emit_s(1)
            emit_delta(1)
            emit_q(1)

            # ---- final: out = (et.T @ wn) / rowsum + wfb ----
            for t in range(nbt):
                po = [
                    psum.tile([P, 512], f32, tag="ps", name=f"po{t}_{oc}")
                    for oc in range(2)
                ]
                prs = psum.tile([P, 1], f32, tag="ps", name=f"prs{t}")
                for hh in range(NT):
                    lhs = et[:, hh, t * P:(t + 1) * P]
                    st = dict(start=(hh == 0), stop=(hh == NT - 1))
                    for oc in range(2):
                        nc.tensor.matmul(
                            po[oc][:], lhs, wn[:, hh, oc * 512:(oc + 1) * 512], **st
                        )
                    nc.tensor.matmul(prs[:], lhs, ones[:], **st)
                recip = small.tile([P, 1], f32, tag="rc", name=f"rc{t}")
                nc.vector.reciprocal(recip[:], prs[:])
                o_st = ost.tile([P, DIM], f32, tag="os", name=f"os{t}")
                for oc in range(2):
                    nc.vector.scalar_tensor_tensor(
                        o_st[:, oc * 512:(oc + 1) * 512],
                        po[oc][:],
                        recip[:],
                        wfb_b[:, oc * 512:(oc + 1) * 512],
                        op0=ALU.mult,
                        op1=ALU.add,
                    )
                nc.sync.dma_start(out=out_h[t * P:(t + 1) * P, :], in_=o_st[:])

    nc.compile()
    return nc


def _host_prep(x, W_slow_w, W_slow_b, W_fast_b, b_core, n_cores):
    """Shard + pre-transpose + cast inputs; returns per-core input maps."""
    wT = np.ascontiguousarray(W_slow_w.T).astype(BF16)
    bk = np.ascontiguousarray(W_slow_b[:DIM]).astype(np.float32)
    bvc = (W_slow_b[DIM:2 * DIM] - W_fast_b).astype(np.float32)
    bq = np.ascontiguousarray(W_slow_b[2 * DIM:3 * DIM]).astype(np.float32)
    blr = np.ascontiguousarray(W_slow_b[3 * DIM:3 * DIM + 1]).astype(np.float32)
    wfb = np.ascontiguousarray(W_fast_b).astype(np.float32)
    in_maps = []
    for c in range(n_cores):
        xs = x[c * b_core:(c + 1) * b_core, :]
        xT = np.ascontiguousarray(xs.T).astype(BF16)
        in_maps.append(
            {"xT": xT, "wT": wT, "bk": bk, "bvc": bvc, "bq": bq, "blr": blr,
             "wfb": wfb}
        )
    return in_maps


_PROGRAM_CACHE = {}


def _get_program(b_core, n_cores=N_CORES):
    key = (b_core, n_cores)
    if key not in _PROGRAM_CACHE:
        _PROGRAM_CACHE[key] = _build_program(b_core, n_cores)
    return _PROGRAM_CACHE[key]


def _run_device(x, W_slow_w, W_slow_b, W_fast_b, trace=False):
    from concourse.bass_utils import run_bass_kernel_spmd

    b_core = x.shape[0] // N_CORES
    nc = _get_program(b_core)
    in_maps = _host_prep(x, W_slow_w, W_slow_b, W_fast_b, b_core, N_CORES)
    res = run_bass_kernel_spmd(nc, in_maps, list(range(N_CORES)), trace=trace)
    out = np.concatenate([res.results[c]["out"] for c in range(N_CORES)], axis=0)
    return out.astype(np.float32), res


def _reference_numpy(x, W_slow_w, W_slow_b, W_fast_w, W_fast_b):
    """Exact fallback (only used if W_fast_w != 0, which the spec never produces)."""
    x = x.astype(np.float64)
    s = x @ W_slow_w.astype(np.float64).T + W_slow_b.astype(np.float64)
    k = s[:, :DIM]
    v = s[:, DIM:2 * DIM]
    q = s[:, 2 * DIM:3 * DIM]
    lr = 1.0 / (1.0 + np.exp(-s[:, -1:]))
    ek = np.exp(k - k.max(axis=1, keepdims=True))
    ak = ek / ek.sum(axis=1, keepdims=True)
    v_bar = ak @ W_fast_w.astype(np.float64).T + W_fast_b.astype(np.float64)
    sigk = 1.0 / (1.0 + np.exp(-k))
    delta = (lr * (v - v_bar)).T @ sigk / x.shape[0]
    w_new = W_fast_w.astype(np.float64) + delta
    eq = np.exp(q - q.max(axis=1, keepdims=True))
    aq = eq / eq.sum(axis=1, keepdims=True)
    return (aq @ w_new.T + W_fast_b.astype(np.float64)).astype(np.float32)


def kernel(x, W_slow_w, W_slow_b, W_fast_w, W_fast_b):
    x = np.asarray(x)
    W_slow_w = np.asarray(W_slow_w)
    W_slow_b = np.asarray(W_slow_b)
    W_fast_w = np.asarray(W_fast_w)
    W_fast_b = np.asarray(W_fast_b)
    if np.any(W_fast_w):
        # Spec guarantees W_fast_w == 0; exact fallback for generality.
        return _reference_numpy(x, W_slow_w, W_slow_b, W_fast_w, W_fast_b)
    out, _ = _run_device(x, W_slow_w, W_slow_b, W_fast_b, trace=False)
    return out


# revision 19
# speedup vs baseline: 1.0664x; 1.0664x over previous
"""DeltaNet fused kernel for 8 TRN2 NeuronCores (Bass/Tile).

Math (reference):
    s  = x @ W_slow_w.T + W_slow_b            [B, 3073]
    k  = s[:, :1024]; v = s[:, 1024:2048]; q = s[:, 2048:3072]
    lr = sigmoid(s[:, 3072])
    v_bar = softmax(k) @ W_fast_w.T + W_fast_b          (W_fast_w == 0 -> v_bar = W_fast_b)
    deltaT[h,o] = sum_b sigmoid(k)[b,h] * (lr*(v - v_bar))[b,o] / B
    out = softmax(q) @ (W_fast_w + delta).T + W_fast_b

Sharding: data-parallel over batch (2048 rows/core). deltaT partials are
AllReduced across the 8 cores (two AllReduces, one per batch half, so the
first overlaps the second half's compute).

Device layout trick: all matmuls use natural layouts (no on-chip transposes):
  - host pre-transposes x (per-shard) and W_slow_w to bf16
  - q is computed TRANSPOSED (qT[h,b]) so exp(qT) is directly the lhsT of the
    final matmul; softmax denominator comes from a ones-vector matmul column
    and is applied as a per-partition output scale.
"""

import os
import sys

for _p in ("/opt/trn_rl_repo", "/root/.axon_site/_ro/trn_rl_repo"):
    if os.path.isdir(_p) and _p not in sys.path:
        sys.path.append(_p)

import numpy as np
import ml_dtypes

BF16 = ml_dtypes.bfloat16

N_CORES = 8
B_FULL = 16384
DIM = 1024          # dim_in == dim_out == dim_hidden
SLOW_OUT = 3 * DIM + 1
P = 128
NT = DIM // P       # 8 tiles along any 1024 dim


def _build_program(b_core: int, n_cores: int = N_CORES):
    """Build the SPMD Bass program (same program on every core)."""
    import concourse.bass as bass
    import concourse.mybir as mybir
    import concourse.tile as tile
    from concourse import bacc

    f32 = mybir.dt.float32
    bf16 = mybir.dt.bfloat16
    AF = mybir.ActivationFunctionType
    ALU = mybir.AluOpType

    assert b_core % (2 * P) == 0
    nbt = b_core // P                 # b-tiles per core
    # asymmetric halves: the larger second half's q-phase hides the second
    # AllReduce's ~50us cost
    nbt1 = max(2, nbt // 4)
    nbt2 = nbt - nbt1
    bh = [nbt1 * P, nbt2 * P]
    off = [0, nbt1 * P]
    nbt_h = [nbt1, nbt2]

    def chunks(width):
        lo = 0
        while lo < width:
            hi = min(lo + 512, width)
            yield lo, hi
            lo = hi

    nc = bacc.Bacc(
        "TRN2",
        target_bir_lowering=False,
        debug=False,
        num_devices=n_cores,
    )

    # ---- kernel I/O ----
    xT_h = nc.dram_tensor("xT", [DIM, b_core], bf16, kind="ExternalInput")
    wT_h = nc.dram_tensor("wT", [DIM, SLOW_OUT], bf16, kind="ExternalInput")
    bk_h = nc.dram_tensor("bk", [DIM], f32, kind="ExternalInput")     # W_slow_b[:1024]
    bvc_h = nc.dram_tensor("bvc", [DIM], f32, kind="ExternalInput")   # W_slow_b[1024:2048] - W_fast_b
    bq_h = nc.dram_tensor("bq", [DIM], f32, kind="ExternalInput")     # W_slow_b[2048:3072]
    blr_h = nc.dram_tensor("blr", [1], f32, kind="ExternalInput")     # W_slow_b[3072]
    wfb_h = nc.dram_tensor("wfb", [DIM], f32, kind="ExternalInput")   # W_fast_b
    out_h = nc.dram_tensor("out", [b_core, DIM], f32, kind="ExternalOutput")

    inv_b = 1.0 / float(b_core * n_cores)

    with tile.TileContext(nc) as tc:
        with (
            tc.tile_pool(name="persist", bufs=1) as persist,
            tc.tile_pool(name="psum", bufs=8, space="PSUM") as psum,
            tc.tile_pool(name="tmp", bufs=4) as tmp,
            tc.tile_pool(name="small", bufs=6) as small,
            tc.tile_pool(name="ost", bufs=2) as ost,
            tc.tile_pool(name="dst", bufs=2) as dstp,
            tc.tile_pool(name="arl", bufs=2) as arl,
            tc.tile_pool(name="dram", bufs=1, space="DRAM") as dram,
        ):
            # ---- persistent SBUF tensors ----
            # wT split into separate tiles per column group so the first
            # s-matmuls only depend on their own group's loads (Tile tracks
            # dependencies per tile object)
            wk = [persist.tile([P, NT, 512], bf16, name=f"wk{c}") for c in range(2)]
            wv = [persist.tile([P, NT, 512], bf16, name=f"wv{c}") for c in range(2)]
            wlr = persist.tile([P, NT, 1], bf16, name="wlr")
            wq = persist.tile([P, NT, DIM], bf16, name="wq")
            xt = persist.tile([P, NT, max(bh)], bf16, name="xt")     # one half at a time
            et = persist.tile([P, NT, b_core], bf16, name="et")      # exp(qT), full batch
            sigk = persist.tile([P, max(nbt_h), DIM], bf16, name="sigk")
            u = persist.tile([P, max(nbt_h), DIM], bf16, name="u")
            wn = persist.tile([P, NT, DIM], bf16, name="wn")         # W_new.T
            bk_b = persist.tile([P, DIM], f32, name="bk_b")
            bvc_b = persist.tile([P, DIM], f32, name="bvc_b")
            wfb_b = persist.tile([P, DIM], f32, name="wfb_b")
            bq_c = persist.tile([P, NT], f32, name="bq_c")
            blr_c = persist.tile([P, 1], f32, name="blr_c")
            ones = persist.tile([P, 1], bf16, name="ones")

            # ---- DRAM bounce buffers for the two AllReduces ----
            ar_in = [
                dram.tile([DIM, DIM], bf16, name=f"ar_in{h}") for h in range(2)
            ]
            ar_out = [
                dram.tile([DIM, DIM], bf16, name=f"ar_out{h}", addr_space="Shared")
                for h in range(2)
            ]

            # ---- constants / weights ----
            nc.vector.memset(ones[:], 1.0)
            # broadcast biases across partitions via K=1 ones-matmuls (PE is
            # idle at startup; saves 1.5 MiB of broadcast-DMA in the critical
            # startup window)
            ones_row = persist.tile([1, P], f32, name="ones_row")
            nc.vector.memset(ones_row[:], 1.0)
            for bi, (bias_dst, bias_src) in enumerate(
                ((bk_b, bk_h), (bvc_b, bvc_h), (wfb_b, wfb_h))
            ):
                for c in range(2):
                    brow = tmp.tile([1, 512], f32, tag="kv", name=f"br{bi}_{c}")
                    nc.gpsimd.dma_start(
                        out=brow[:],
                        in_=bass.AP(tensor=bias_src, offset=c * 512,
                                    ap=[[0, 1], [1, 512]]),
                    )
                    pb = psum.tile([P, 512], f32, tag="ps", name=f"pb{bi}_{c}")
                    nc.tensor.matmul(
                        pb[:], ones_row[:, :], brow[:], start=True, stop=True
                    )
                    nc.vector.tensor_copy(bias_dst[:, c * 512:(c + 1) * 512], pb[:])
            # bq_c[p, i] = bq[i*128 + p]
            nc.gpsimd.dma_start(
                out=bq_c[:],
                in_=bass.AP(tensor=bq_h, offset=0, ap=[[1, P], [P, NT]]),
            )
            nc.gpsimd.dma_start(
                out=blr_c[:],
                in_=bass.AP(tensor=blr_h, offset=0, ap=[[0, P], [1, 1]]),
            )
            # xT (half 0) first so the s-phase can start ASAP, then wT groups
            # in consumption order (lr column first: tiny).
            for i in range(NT):
                nc.sync.dma_start(
                    out=xt[:, i, 0:bh[0]], in_=xT_h[i * P:(i + 1) * P, 0:bh[0]]
                )
            for c in range(2):
                for i in range(NT):
                    nc.sync.dma_start(
                        out=wk[c][:, i, :],
                        in_=wT_h[i * P:(i + 1) * P, c * 512:(c + 1) * 512],
                    )
            for i in range(NT):
                nc.sync.dma_start(
                    out=wlr[:, i, :], in_=wT_h[i * P:(i + 1) * P, 3 * DIM:SLOW_OUT]
                )
            for c in range(2):
                for i in range(NT):
                    nc.sync.dma_start(
                        out=wv[c][:, i, :],
                        in_=wT_h[i * P:(i + 1) * P, DIM + c * 512:DIM + (c + 1) * 512],
                    )
            for i in range(NT):
                nc.sync.dma_start(
                    out=wq[:, i, :], in_=wT_h[i * P:(i + 1) * P, 2 * DIM:3 * DIM]
                )

            def emit_s(half):
                """k / v / lr per b-tile; fills sigk and u for this half."""
                for t in range(nbt_h[half]):
                    ps = [
                        psum.tile([P, 512], f32, tag="ps", name=f"ps{half}_{t}_{c}")
                        for c in range(4)
                    ]
                    plr = psum.tile([P, 1], f32, tag="ps", name=f"plr{half}_{t}")
                    for i in range(NT):
                        lhs = xt[:, i, t * P:(t + 1) * P]
                        st = dict(start=(i == 0), stop=(i == NT - 1))
                        for c in range(2):
                            nc.tensor.matmul(ps[c][:], lhs, wk[c][:, i, :], **st)
                        for c in range(2):
                            nc.tensor.matmul(ps[2 + c][:], lhs, wv[c][:, i, :], **st)
                        nc.tensor.matmul(plr[:], lhs, wlr[:, i, :], **st)
                    # lr = sigmoid(plr + blr) / B
                    lr_s = small.tile([P, 1], f32, tag="lr", name=f"lr{half}_{t}")
                    nc.scalar.activation(lr_s[:], plr[:], AF.Sigmoid, bias=blr_c[:, 0:1])
                    nc.vector.tensor_scalar_mul(lr_s[:], lr_s[:], inv_b)
                    for c in range(2):
                        # sigk = sigmoid(k + bk)
                        ktmp = tmp.tile([P, 512], f32, tag="kv", name=f"kt{half}_{t}_{c}")
                        nc.vector.tensor_add(ktmp[:], ps[c][:], bk_b[:, c * 512:(c + 1) * 512])
                        nc.scalar.activation(
                            sigk[:, t, c * 512:(c + 1) * 512], ktmp[:], AF.Sigmoid
                        )
                        # u = lr/B * (v + (bv - wfb))
                        vtmp = tmp.tile([P, 512], f32, tag="kv", name=f"vt{half}_{t}_{c}")
                        nc.vector.tensor_add(
                            vtmp[:], ps[2 + c][:], bvc_b[:, c * 512:(c + 1) * 512]
                        )
                        nc.scalar.activation(
                            u[:, t, c * 512:(c + 1) * 512], vtmp[:], AF.Copy,
                            scale=lr_s[:],
                        )

            def emit_delta(half):
                """deltaT_half[h, o] = sum_b sigk * u, staged + AllReduced.

                For the second half the first AllReduce's output is folded
                into the input (scaled by 1/n_cores so the sum over cores
                adds it exactly once) — W_newT is then just ar_out[1]."""
                nb = nbt_h[half]
                for hh in range(NT):
                    pd = [
                        psum.tile([P, 512], f32, tag="ps", name=f"pd{half}_{hh}_{oc}")
                        for oc in range(2)
                    ]
                    for t in range(nb):
                        st = dict(start=(t == 0), stop=(t == nb - 1))
                        lhs = sigk[:, t, hh * P:(hh + 1) * P]
                        for oc in range(2):
                            nc.tensor.matmul(
                                pd[oc][:], lhs, u[:, t, oc * 512:(oc + 1) * 512], **st
                            )
                    dstage = dstp.tile([P, DIM], bf16, tag="ds", name=f"ds{half}_{hh}")
                    if half == 0:
                        for oc in range(2):
                            nc.vector.tensor_copy(
                                dstage[:, oc * 512:(oc + 1) * 512], pd[oc][:]
                            )
                    else:
                        a0 = arl.tile([P, DIM], bf16, tag="ar", name=f"a0_{hh}")
                        nc.sync.dma_start(
                            out=a0[:], in_=ar_out[0][hh * P:(hh + 1) * P, :]
                        )
                        for oc in range(2):
                            nc.vector.scalar_tensor_tensor(
                                dstage[:, oc * 512:(oc + 1) * 512],
                                a0[:, oc * 512:(oc + 1) * 512],
                                1.0 / n_cores,
                                pd[oc][:],
                                op0=ALU.mult,
                                op1=ALU.add,
                            )
                    nc.sync.dma_start(
                        out=ar_in[half][hh * P:(hh + 1) * P, :], in_=dstage[:]
                    )
                nc.gpsimd.collective_compute(
                    "AllReduce",
                    mybir.AluOpType.add,
                    replica_groups=[list(range(n_cores))],
                    ins=[ar_in[half][:, :]],
                    outs=[ar_out[half][:, :]],
                )
                if half == 1:
                    for hh in range(NT):
                        nc.sync.dma_start(
                            out=wn[:, hh, :], in_=ar_out[1][hh * P:(hh + 1) * P, :]
                        )

            def emit_q(half):
                """et = exp(qT + bq) (transposed layout)."""
                for hh in range(NT):
                    for lo, hi in chunks(bh[half]):
                        w = hi - lo
                        pq = psum.tile(
                            [P, 512], f32, tag="ps", name=f"pq{half}_{hh}_{lo}"
                        )
                        for i in range(NT):
                            nc.tensor.matmul(
                                pq[:, 0:w],
                                wq[:, i, hh * P:(hh + 1) * P],
                                xt[:, i, lo:hi],
                                start=(i == 0),
                                stop=(i == NT - 1),
                            )
                        nc.scalar.activation(
                            et[:, hh, off[half] + lo:off[half] + hi],
                            pq[:, 0:w],
                            AF.Exp,
                            bias=bq_c[:, hh:hh + 1],
                        )

            # Both q-phases run AFTER the second AllReduce's doorbell so
            # ~68us of PE work hides the collective's 40-75us cost. xT half 0
            # is reloaded for the late q0 (the reload hides under q1).
            emit_s(0)
            emit_delta(0)
            for i in range(NT):
                nc.sync.dma_start(
                    out=xt[:, i, 0:bh[1]],
                    in_=xT_h[i * P:(i + 1) * P, off[1]:off[1] + bh[1]],
                )
            emit_s(1)
            emit_delta(1)
            emit_q(1)
            for i in range(NT):
                nc.sync.dma_start(
                    out=xt[:, i, 0:bh[0]], in_=xT_h[i * P:(i + 1) * P, 0:bh[0]]
                )
            emit_q(0)

            # ---- final: out = (et.T @ wn) / rowsum + wfb ----
            for t in range(nbt):
                po = [
                    psum.tile([P, 512], f32, tag="ps", name=f"po{t}_{oc}")
                    for oc in range(2)
                ]
                prs = psum.tile([P, 1], f32, tag="ps", name=f"prs{t}")
                for hh in range(NT):
                    lhs = et[:, hh, t * P:(t + 1) * P]
                    st = dict(start=(hh == 0), stop=(hh == NT - 1))
                    for oc in range(2):
                        nc.tensor.matmul(
                            po[oc][:], lhs, wn[:, hh, oc * 512:(oc + 1) * 512], **st
                        )
                    nc.tensor.matmul(prs[:], lhs, ones[:], **st)
                recip = small.tile([P, 1], f32, tag="rc", name=f"rc{t}")
                nc.vector.reciprocal(recip[:], prs[:])
                o_st = ost.tile([P, DIM], f32, tag="os", name=f"os{t}")
                for oc in range(2):
                    nc.vector.scalar_tensor_tensor(
                        o_st[:, oc * 512:(oc + 1) * 512],
                        po[oc][:],
                        recip[:],
                        wfb_b[:, oc * 512:(oc + 1) * 512],
                        op0=ALU.mult,
                        op1=ALU.add,
                    )
                nc.sync.dma_start(out=out_h[t * P:(t + 1) * P, :], in_=o_st[:])

    nc.compile()
    return nc


def _host_prep(x, W_slow_w, W_slow_b, W_fast_b, b_core, n_cores):
    """Shard + pre-transpose + cast inputs; returns per-core input maps."""
    wT = np.ascontiguousarray(W_slow_w.T).astype(BF16)
    bk = np.ascontiguousarray(W_slow_b[:DIM]).astype(np.float32)
    bvc = (W_slow_b[DIM:2 * DIM] - W_fast_b).astype(np.float32)
    bq = np.ascontiguousarray(W_slow_b[2 * DIM:3 * DIM]).astype(np.float32)
    blr = np.ascontiguousarray(W_slow_b[3 * DIM:3 * DIM + 1]).astype(np.float32)
    wfb = np.ascontiguousarray(W_fast_b).astype(np.float32)
    in_maps = []
    for c in range(n_cores):
        xs = x[c * b_core:(c + 1) * b_core, :]
        xT = np.ascontiguousarray(xs.T).astype(BF16)
        in_maps.append(
            {"xT": xT, "wT": wT, "bk": bk, "bvc": bvc, "bq": bq, "blr": blr,
             "wfb": wfb}
        )
    return in_maps


_PROGRAM_CACHE = {}


def _get_program(b_core, n_cores=N_CORES):
    key = (b_core, n_cores)
    if key not in _PROGRAM_CACHE:
        _PROGRAM_CACHE[key] = _build_program(b_core, n_cores)
    return _PROGRAM_CACHE[key]


def _run_device(x, W_slow_w, W_slow_b, W_fast_b, trace=False):
    from concourse.bass_utils import run_bass_kernel_spmd

    b_core = x.shape[0] // N_CORES
    nc = _get_program(b_core)
    in_maps = _host_prep(x, W_slow_w, W_slow_b, W_fast_b, b_core, N_CORES)
    res = run_bass_kernel_spmd(nc, in_maps, list(range(N_CORES)), trace=trace)
    out = np.concatenate([res.results[c]["out"] for c in range(N_CORES)], axis=0)
    return out.astype(np.float32), res


def _reference_numpy(x, W_slow_w, W_slow_b, W_fast_w, W_fast_b):
    """Exact fallback (only used if W_fast_w != 0, which the spec never produces)."""
    x = x.astype(np.float64)
    s = x @ W_slow_w.astype(np.float64).T + W_slow_b.astype(np.float64)
    k = s[:, :DIM]
    v = s[:, DIM:2 * DIM]
    q = s[:, 2 * DIM:3 * DIM]
    lr = 1.0 / (1.0 + np.exp(-s[:, -1:]))
    ek = np.exp(k - k.max(axis=1, keepdims=True))
    ak = ek / ek.sum(axis=1, keepdims=True)
    v_bar = ak @ W_fast_w.astype(np.float64).T + W_fast_b.astype(np.float64)
    sigk = 1.0 / (1.0 + np.exp(-k))
    delta = (lr * (v - v_bar)).T @ sigk / x.shape[0]
    w_new = W_fast_w.astype(np.float64) + delta
    eq = np.exp(q - q.max(axis=1, keepdims=True))
    aq = eq / eq.sum(axis=1, keepdims=True)
    return (aq @ w_new.T + W_fast_b.astype(np.float64)).astype(np.float32)


def kernel(x, W_slow_w, W_slow_b, W_fast_w, W_fast_b):
    x = np.asarray(x)
    W_slow_w = np.asarray(W_slow_w)
    W_slow_b = np.asarray(W_slow_b)
    W_fast_w = np.asarray(W_fast_w)
    W_fast_b = np.asarray(W_fast_b)
    if np.any(W_fast_w):
        # Spec guarantees W_fast_w == 0; exact fallback for generality.
        return _reference_numpy(x, W_slow_w, W_slow_b, W_fast_w, W_fast_b)
    out, _ = _run_device(x, W_slow_w, W_slow_b, W_fast_b, trace=False)
    return out
